# revision 23
# baseline (speedup 1.0000x reference)
"""Trainium2 Bass kernel for nn_CaserQueryEncoder.

Model (B=1024, L=50, D=128, NV=8, NH=16):
  P_u = user_emb[user_ids]                                   [B, D]
  E   = item_emb[item_seq]                                   [B, L, D]
  o_v = einsum('btd,vt->bvd', E, Wv) + bv                    [B, NV*D]
  conv[b,i,j,t] = sum_{dt<=i} <E[b, t+dt, :], Wh[i,j,dt,:]>  (Wh zero for dt>i)
  o_h[b,i,j] = max over valid t (t <= 49-i) of relu(conv + bh)
  z = relu([o_v, o_h] @ fc_W + fc_b)                         [B, D]
  out = [z, P_u]                                             [B, 2D]

Strategy: pure data parallel, 128 batch rows per core x 8 cores, no
collectives. Per core:
  - indirect-DMA gather of E (6400 rows) and P_u; each gathered [b, d]
    block is cast to bf16 on the scalar engine, PE-transposed (bf16,
    1 cyc/row) and copied into ET[d, b, t'] (t' padded to 64 with zeros
    = the conv zero padding).
  - ALL matmul operands are bf16 (PSUM accumulation stays fp32): bf16
    enables the PE fast-weight-load path (f32r disables it) and halves
    the 14MB conv weight stream to 7MB.
  - horizontal conv as PSUM-accumulated shifted matmuls: heights packed
    8 per chunk (x16 filters = M=128 weight columns); for each tap dt the
    rhs is ET shifted by dt in t'; PSUM accumulates over dt. Invalid
    (height, t) positions get an additive -1e30 mask before the max-
    reduce. max(relu(x+b)) == relu(max(x)+b), so relu+bias happen after
    the max on the scalar engine.
  - conv weights stream as 28 tap-group DMAs (<=8 taps each) issued in
    the order the gather chase needs them, so the first chunks' weights
    land ~2µs after kernel start instead of ~30µs.
  - vertical conv never materialized: since o_v enters the fc linearly,
    G[t,d,k] = sum_v Wv[v,t]*fc_W[v*128+d,k] is precomputed on host and
    E @ G is added straight into the fc accumulation PSUM.
  - fc bias added via a K=1 ones-matmul.
"""

import math
import os
import sys
from contextlib import ExitStack

import numpy as np

sys.path.insert(0, "/opt/trn_rl_repo")

import ml_dtypes

import concourse.bass as bass
import concourse.tile as tile
from concourse import mybir
from concourse.bass import IndirectOffsetOnAxis
from concourse.bass_utils import run_bass_kernel_spmd
from concourse.masks import make_identity
from concourse.vector_clock import ScopedClock


def _patch_tile_drain():
    """This container's walrus codegen only accepts one sync-wait per Drain
    (CTRL_NO_STRUCT); Tile's kernel-tail drain carries one wait per live
    semaphore. Split the waits across a chain of drains, one wait each."""
    if getattr(tile.TileContext, "_drain_split_patched", False):
        return

    def _patched(self, tick_clock, wait_clock):
        nc = self.nc
        probe = nc.sync.drain()
        wait_clock.add_sem_waits(
            probe.ins, ScopedClock({None: tick_clock.global_clock}))
        nc.all_engine_barrier()
        popped = nc._tile_sem_poison_stack.pop()
        assert popped is self._sem_poison
        nc.clear_and_free_semaphores(list(self.sems.allocated().values()))
        nc.all_engine_barrier()

    tile.TileContext._drain_and_barrier = _patched
    tile.TileContext._drain_split_patched = True


_patch_tile_drain()


def _split_json_waits(j, max_waits=1):
    """This walrus codegen accepts at most one sync-wait per instruction.
    Hoist extra waits onto wait-only EventSemaphore instructions inserted
    just before the offender on the same engine queue."""
    n = 0
    for fn in j["functions"]:
        for blk in fn["blocks"]:
            out = []
            for inst in blk["instructions"]:
                si = inst.get("sync_info")
                waits = (si or {}).get("on_wait") or []
                if len(waits) > max_waits:
                    for k, w in enumerate(waits[:-max_waits]):
                        out.append({
                            "debug": inst.get("debug", 0),
                            "engine": inst["engine"],
                            "ins": [], "outs": [],
                            "name": f"{inst['name']}_wsplit{k}",
                            "opcode": "EventSemaphore",
                            "sync_info": {"on_update": [], "on_wait": [w]},
                        })
                        n += 1
                    si["on_wait"] = waits[-max_waits:]
                out.append(inst)
            blk["instructions"] = out
    return n


def _install_wait_splitter(nc):
    import json as _json

    orig = nc.to_json_bytes

    def patched():
        j = _json.loads(orig())
        _split_json_waits(j)
        return _json.dumps(j).encode()

    nc.to_json_bytes = patched

B = 1024
L = 50
D = 128
NV = 8
NH = 16
NU = 100000
NI = 500000
NCORES = 8
BLOC = B // NCORES          # 128 batch rows per core
TP = 64                     # t' pitch in ET (>= max dt + max Nt = 56)
NEG = -1.0e30
FC_IN = NV * D + NH * L     # 1824
NOUT = 2 * D                # 256
WGRP = 8                    # taps per weight-DMA group

# Height-chunk table: heights [8u, 8u+nh) packed as m2 = 16*(i-8u)+j.
# ndt taps accumulate in PSUM; Nt is the t-window (valid-t of the chunk's
# shortest filter); Nb batch rows per matmul so that Nb*Nt <= 512.
CHUNKS = []
_base = 0
for _u in range(7):
    _i0 = 8 * _u
    _nh = min(8, L - _i0)
    _ndt = min(_i0 + 8, L)
    _nt = L - _i0
    _nb = min(BLOC, 512 // _nt)
    _nblk = math.ceil(BLOC / _nb)
    CHUNKS.append(dict(i0=_i0, nh=_nh, ndt=_ndt, nt=_nt, nb=_nb,
                       nblk=_nblk, base=_base))
    _base += _ndt
NWTILES = _base             # 218 weight tiles of [d=128, m2=128]

# Weight-DMA groups: chunk u's taps split into ceil(ndt/WGRP) slices;
# group (u, k) covers taps [k*WGRP, min((k+1)*WGRP, ndt)). It is first
# needed when tap k*WGRP becomes runnable, i.e. at gather column
# min(k*WGRP + nt - 1, L-1). Stream the DMAs in need order.
WGROUPS = []                # (need_col, u, k, dt0, dt1)
for _u, _ch in enumerate(CHUNKS):
    for _k in range(math.ceil(_ch["ndt"] / WGRP)):
        _dt0 = _k * WGRP
        _dt1 = min(_dt0 + WGRP, _ch["ndt"])
        _need = min(_dt0 + _ch["nt"] - 1, L - 1)
        WGROUPS.append((_need, _u, _k, _dt0, _dt1))
WGROUPS.sort(key=lambda g: (g[0], -g[1]))
# trigger each group's DMA a few gather slots before it is needed
WTRIG = {}
for _g in WGROUPS:
    WTRIG.setdefault(max(0, _g[0] - 6), []).append(_g)

_NC_CACHE = None

# Set BASS_KERNEL_TRACE=1 to profile; exec time lands in LAST_RESULTS.
LAST_RESULTS = None


def _build_nc():
    f32 = mybir.dt.float32
    bf16 = mybir.dt.bfloat16
    i32 = mybir.dt.int32
    X = mybir.AxisListType.X
    Copy = mybir.ActivationFunctionType.Copy

    nc = bass.Bass()
    seq_t = nc.dram_tensor("seq_idx", [BLOC, L], i32, kind="ExternalInput")
    uid_t = nc.dram_tensor("uid_idx", [BLOC, 1], i32, kind="ExternalInput")
    item_t = nc.dram_tensor("item_emb", [NI, D], f32, kind="ExternalInput")
    user_t = nc.dram_tensor("user_emb", [NU, D], f32, kind="ExternalInput")
    whp_t = nc.dram_tensor("whp", [D, NWTILES * 128], bf16, kind="ExternalInput")
    g_t = nc.dram_tensor("g", [D, L * D], bf16, kind="ExternalInput")
    fcwh_t = nc.dram_tensor("fcwh", [128, 7 * D], bf16, kind="ExternalInput")
    masks_t = nc.dram_tensor("masks", [128, 7 * 512], f32, kind="ExternalInput")
    bh_t = nc.dram_tensor("bh_p", [128, 7], f32, kind="ExternalInput")
    fcb_t = nc.dram_tensor("fcb", [1, D], bf16, kind="ExternalInput")
    out_t = nc.dram_tensor("out", [BLOC, NOUT], f32, kind="ExternalOutput")

    # conv matmul (u, blk, dt) becomes runnable once ET column
    # min(dt + Nt - 1, L-1) is gathered (t' >= L is the zero pad).
    # These six groups chase the gather stream; the rest run after it.
    PHASE_A = [(6, 0), (5, 0), (5, 1), (5, 2), (4, 0), (4, 1)]

    with ExitStack() as ctx:
        tc = ctx.enter_context(tile.TileContext(nc))
        const = ctx.enter_context(tc.tile_pool(name="const", bufs=1))
        egath = ctx.enter_context(tc.tile_pool(name="egath", bufs=16))
        ebfp = ctx.enter_context(tc.tile_pool(name="ebfp", bufs=4))
        gpool = ctx.enter_context(tc.tile_pool(name="gpool", bufs=8))
        etp = ctx.enter_context(tc.tile_pool(name="etp", bufs=1))
        wpool = ctx.enter_context(tc.tile_pool(name="wpool", bufs=1))
        ohp = ctx.enter_context(tc.tile_pool(name="ohp", bufs=1))
        misc = ctx.enter_context(tc.tile_pool(name="misc", bufs=1))
        tpsum = ctx.enter_context(tc.tile_pool(name="tpsum", bufs=1, space="PSUM"))
        cpsum = ctx.enter_context(tc.tile_pool(name="cpsum", bufs=6, space="PSUM"))
        zpsum = ctx.enter_context(tc.tile_pool(name="zpsum", bufs=1, space="PSUM"))

        # --- constants (sync ring: small, then the 50 g slices) ---
        seq_sb = const.tile([BLOC, L], i32)
        nc.sync.dma_start(out=seq_sb[:], in_=seq_t[:])
        uid_sb = const.tile([BLOC, 1], i32)
        nc.sync.dma_start(out=uid_sb[:], in_=uid_t[:])
        ident = const.tile([128, 128], bf16)
        make_identity(nc, ident[:])
        fcwh_sb = const.tile([128, 7 * D], bf16)
        nc.sync.dma_start(out=fcwh_sb[:], in_=fcwh_t[:])
        mask_sb = const.tile([128, 7 * 512], f32)
        nc.sync.dma_start(out=mask_sb[:], in_=masks_t[:])
        bh_sb = const.tile([128, 7], f32)
        nc.sync.dma_start(out=bh_sb[:], in_=bh_t[:])
        fcb_sb = const.tile([1, D], bf16)
        nc.sync.dma_start(out=fcb_sb[:], in_=fcb_t[:])
        ones_sb = const.tile([1, BLOC], bf16)
        nc.vector.memset(ones_sb[:], 1.0)
        zline = const.tile([D, 1], bf16)
        nc.vector.memset(zline[:], 0.0)
        zfill = const.tile([128, BLOC], bf16)
        nc.vector.memset(zfill[:], 0.0)

        # --- conv weights: one resident tile, streamed per tap-group on the
        # scalar ring, in chase-need order (interleaved into the t-loop) ---
        whp_sb = wpool.tile([D, NWTILES * 128], bf16, name="whp_sb")

        def load_wgroup(g):
            _, u, _, dt0, dt1 = g
            base = CHUNKS[u]["base"]
            c0, c1 = (base + dt0) * 128, (base + dt1) * 128
            nc.scalar.dma_start(out=whp_sb[:, c0:c1], in_=whp_t[:, c0:c1])

        # --- ET[d, b, t'], zero pad for t' >= L ---
        et = etp.tile([D, BLOC, TP], bf16)
        nc.vector.tensor_copy(out=et[:, :, L:TP],
                              in_=zline[:].to_broadcast([D, BLOC, TP - L]))

        # --- fc accumulation PSUM [b, k]; group closes on last o_h matmul.
        # The opening bias matmul is emitted inside the t-loop (after the
        # first transpose) so a slow fcb load can't stall the PE queue head.
        zp = zpsum.tile([BLOC, D], f32)

        # conv emission bookkeeping
        chase = {}
        fc_pending = []
        for u, blk in PHASE_A:
            nt = CHUNKS[u]["nt"]
            for dt in range(CHUNKS[u]["ndt"]):
                chase.setdefault(min(dt + nt - 1, L - 1), []).append((u, blk, dt))
        psum_tiles = {}
        blocks_left = [ch["nblk"] for ch in CHUNKS]
        oh_tiles = {}

        def get_ohu(u):
            if u not in oh_tiles:
                oh_tiles[u] = ohp.tile([128, BLOC], bf16, tag=f"oh{u}",
                                       name=f"oh{u}")
            return oh_tiles[u]

        def emit_conv_mm(u, blk, dt):
            ch = CHUNKS[u]
            nt, nb, ndt = ch["nt"], ch["nb"], ch["ndt"]
            b0 = blk * nb
            nbb = min(nb, BLOC - b0)
            n = nbb * nt
            key = (u, blk)
            if key not in psum_tiles:
                while len(fc_pending) > 1:
                    uu = fc_pending.pop(0)
                    nc.tensor.matmul(out=zp[:], lhsT=oh_tiles[uu][:],
                                     rhs=fcwh_sb[:, uu * D:(uu + 1) * D],
                                     start=False, stop=False)
                psum_tiles[key] = cpsum.tile([128, 512], f32, tag="cps",
                                             name=f"cps_{u}_{blk}")
            ps = psum_tiles[key]
            # Tap dt only feeds heights i >= dt, whose valid t stops at
            # 49 - max(dt, i0): shrink the t-window for late taps. The
            # skipped cells are all masked to -1e30 before the reduce, so
            # the output is bit-identical. Saves ~4% of conv PE rows.
            nt_eff = min(nt, L - max(dt, ch["i0"]))
            out_ap = (ps[:, :n] if nt_eff == nt else
                      ps[:, :n].rearrange("p (b t) -> p b t", t=nt)[:, :, :nt_eff])
            nc.tensor.matmul(
                out=out_ap,
                lhsT=whp_sb[:, (ch["base"] + dt) * 128:(ch["base"] + dt + 1) * 128],
                rhs=et[:, b0:b0 + nbb, dt:dt + nt_eff],
                start=(dt == 0), stop=(dt == ndt - 1))
            if dt == ndt - 1:
                nc.vector.tensor_tensor(
                    out=ps[:, :n], in0=ps[:, :n],
                    in1=mask_sb[:, u * 512:u * 512 + n],
                    op=mybir.AluOpType.add)
                nc.vector.reduce_max(
                    out=get_ohu(u)[:, b0:b0 + nbb],
                    in_=ps[:, :n].rearrange("p (b t) -> p b t", t=nt),
                    axis=X)
                del psum_tiles[key]
                blocks_left[u] -= 1
                if blocks_left[u] == 0:
                    ohu = get_ohu(u)
                    nc.scalar.activation(ohu[:], ohu[:],
                                         mybir.ActivationFunctionType.Relu,
                                         bias=bh_sb[:, u:u + 1])
                    fc_pending.append(u)

        # --- the chase loop: gather -> bf16 cast -> transpose -> copy ->
        # G matmul, with ready conv matmuls interleaved into the PE stream
        # and weight-group DMAs interleaved into the scalar queue ---
        g_tiles = {}

        def emit_g_mm(t):
            nc.tensor.matmul(out=zp[:], lhsT=et[:, :, t], rhs=g_tiles.pop(t),
                             start=False, stop=False)

        for t in range(L):
            for g in WTRIG.get(t, ()):
                load_wgroup(g)
            e_t = egath.tile([BLOC, D], f32, tag="eg")
            nc.gpsimd.indirect_dma_start(
                out=e_t[:], out_offset=None, in_=item_t[:],
                in_offset=IndirectOffsetOnAxis(ap=seq_sb[:, t:t + 1], axis=0))
            ebf = ebfp.tile([BLOC, D], bf16, tag="ebf")
            nc.scalar.activation(ebf[:], e_t[:], Copy)
            tp = tpsum.tile([128, 128], bf16, tag="tp")
            nc.tensor.transpose(out=tp[:], in_=ebf[:], identity=ident[:])
            nc.vector.tensor_copy(out=et[:, :, t], in_=tp[:])
            gt = gpool.tile([D, D], bf16, tag="g", name=f"g{t}")
            nc.sync.dma_start(out=gt[:], in_=g_t[:, t * D:(t + 1) * D])
            g_tiles[t] = gt
            if t == 0:
                nc.tensor.matmul(out=zp[:], lhsT=ones_sb[:], rhs=fcb_sb[:],
                                 start=True, stop=False)
            else:
                emit_g_mm(t - 1)
            for (u, blk, dt) in chase.get(t, ()):
                emit_conv_mm(u, blk, dt)
            # Zero-contribution fillers (exact +0 into the open zp group)
            # keep the PE gaplessly busy through the gather window so HAM
            # ramps to full clock early and stays there.
            if t >= 1:
                nfill = 12 if t < 9 else (6 if t < 17 else (4 if t < 25 else 2))
                for _ in range(nfill):
                    nc.tensor.matmul(out=zp[:], lhsT=zfill[:],
                                     rhs=ident[:], start=False, stop=False)
        emit_g_mm(L - 1)

        # --- the P_u gather (off the gather window's critical path) ---
        pu_sb = misc.tile([BLOC, D], f32, tag="pu")
        nc.gpsimd.indirect_dma_start(
            out=pu_sb[:], out_offset=None, in_=user_t[:],
            in_offset=IndirectOffsetOnAxis(ap=uid_sb[:, :1], axis=0))
        nc.sync.dma_start(out=out_t[:, D:NOUT], in_=pu_sb[:])

        # --- remaining conv chunks, block-sequential: each block's reduce
        # overlaps the next block's matmuls on a different PSUM bank ---
        done_a = set(PHASE_A)
        for u in [4, 3, 2, 1, 0]:
            for blk in range(CHUNKS[u]["nblk"]):
                if (u, blk) in done_a:
                    continue
                for dt in range(CHUNKS[u]["ndt"]):
                    emit_conv_mm(u, blk, dt)

        # --- remaining o_h fc matmuls ---
        for i, u in enumerate(fc_pending):
            nc.tensor.matmul(out=zp[:], lhsT=oh_tiles[u][:],
                             rhs=fcwh_sb[:, u * D:(u + 1) * D],
                             start=False, stop=(i == len(fc_pending) - 1))

        z_sb = misc.tile([BLOC, D], f32, tag="z")
        nc.scalar.activation(z_sb[:], zp[:], mybir.ActivationFunctionType.Relu)
        nc.sync.dma_start(out=out_t[:, 0:D], in_=z_sb[:])

    return nc


def _prep_common(user_emb, item_emb, Wv, bv, Wh, bh, fc_W, fc_b):
    f = np.float32
    b16 = ml_dtypes.bfloat16
    item_emb = np.ascontiguousarray(np.asarray(item_emb, f))
    user_emb = np.ascontiguousarray(np.asarray(user_emb, f))
    Wh = np.asarray(Wh, f)          # [L, NH, L, D], zero for dt > i
    Wv = np.asarray(Wv, f)          # [NV, L]
    bv = np.asarray(bv, f)
    bh = np.asarray(bh, f)          # [L, NH]
    fc_W = np.asarray(fc_W, f)      # [FC_IN, D]
    fc_b = np.asarray(fc_b, f)

    whp = np.zeros((D, NWTILES * 128), f)
    masks = np.full((128, 7 * 512), 0.0, f)
    fcwh = np.zeros((128, 7 * D), f)
    bh_p = np.zeros((128, 7), f)
    fcw_h = fc_W[NV * D:]           # [800, D]
    for u, ch in enumerate(CHUNKS):
        i0, nh, ndt, nt, nb = ch["i0"], ch["nh"], ch["ndt"], ch["nt"], ch["nb"]
        base = ch["base"]
        wu = Wh[i0:i0 + nh]         # [nh, NH, L, D]
        for dt in range(ndt):
            blkw = wu[:, :, dt, :].reshape(nh * NH, D)
            whp[:, (base + dt) * 128:(base + dt) * 128 + nh * NH] = blkw.T
        m = np.full((128, nb * nt), NEG, f)
        for mm in range(nh * NH):
            i = i0 + mm // NH
            vt = min(L - i, nt)
            row = np.full((nt,), NEG, f)
            row[:vt] = 0.0
            m[mm] = np.tile(row, nb)
        masks[:, u * 512:u * 512 + nb * nt] = m
        fcwh[:nh * NH, u * D:(u + 1) * D] = fcw_h[u * 128:u * 128 + nh * NH]
        bh_p[:nh * NH, u] = bh[i0:i0 + nh].reshape(nh * NH)

    fcv = fc_W[:NV * D].reshape(NV, D, D)
    g = np.einsum("vt,vdk->tdk", Wv, fcv)            # [L, D, D]
    g = np.ascontiguousarray(g.transpose(1, 0, 2).reshape(D, L * D))
    fcb = (fc_b + np.einsum("v,vdk->k", bv, fcv)).reshape(1, D)

    return dict(item_emb=item_emb, user_emb=user_emb,
                whp=whp.astype(b16), g=g.astype(b16),
                fcwh=fcwh.astype(b16), masks=masks, bh_p=bh_p,
                fcb=fcb.astype(b16))


def make_in_maps(user_ids, item_seq, user_emb, item_emb, Wv, bv, Wh, bh,
                 fc_W, fc_b):
    common = _prep_common(user_emb, item_emb, Wv, bv, Wh, bh, fc_W, fc_b)
    user_ids = np.asarray(user_ids).astype(np.int32).reshape(B, 1)
    item_seq = np.asarray(item_seq).astype(np.int32).reshape(B, L)
    in_maps = []
    for c in range(NCORES):
        m = dict(common)
        m["seq_idx"] = np.ascontiguousarray(item_seq[c * BLOC:(c + 1) * BLOC])
        m["uid_idx"] = np.ascontiguousarray(user_ids[c * BLOC:(c + 1) * BLOC])
        in_maps.append(m)
    return in_maps


def get_nc():
    global _NC_CACHE
    if _NC_CACHE is None:
        _NC_CACHE = _build_nc()
        _install_wait_splitter(_NC_CACHE)
    return _NC_CACHE


def kernel(**inputs) -> np.ndarray:
    global LAST_RESULTS
    in_maps = make_in_maps(**inputs)
    nc = get_nc()
    trace = bool(int(os.environ.get("BASS_KERNEL_TRACE", "0")))
    res = run_bass_kernel_spmd(nc, in_maps, list(range(NCORES)), trace=trace)
    LAST_RESULTS = res
    return np.concatenate([res.results[c]["out"] for c in range(NCORES)], axis=0)


# revision 24
# speedup vs baseline: 1.0010x; 1.0010x over previous
"""Trainium2 Bass kernel for nn_CaserQueryEncoder.

Model (B=1024, L=50, D=128, NV=8, NH=16):
  P_u = user_emb[user_ids]                                   [B, D]
  E   = item_emb[item_seq]                                   [B, L, D]
  o_v = einsum('btd,vt->bvd', E, Wv) + bv                    [B, NV*D]
  conv[b,i,j,t] = sum_{dt<=i} <E[b, t+dt, :], Wh[i,j,dt,:]>  (Wh zero for dt>i)
  o_h[b,i,j] = max over valid t (t <= 49-i) of relu(conv + bh)
  z = relu([o_v, o_h] @ fc_W + fc_b)                         [B, D]
  out = [z, P_u]                                             [B, 2D]

Strategy: pure data parallel, 128 batch rows per core x 8 cores, no
collectives. Per core:
  - indirect-DMA gather of E (6400 rows) and P_u; each gathered [b, d]
    block is cast to bf16 on the scalar engine, PE-transposed (bf16,
    1 cyc/row) and copied into ET[d, b, t'] (t' padded to 64 with zeros
    = the conv zero padding).
  - ALL matmul operands are bf16 (PSUM accumulation stays fp32): bf16
    enables the PE fast-weight-load path (f32r disables it) and halves
    the 14MB conv weight stream to 7MB.
  - horizontal conv as PSUM-accumulated shifted matmuls: heights packed
    8 per chunk (x16 filters = M=128 weight columns); for each tap dt the
    rhs is ET shifted by dt in t'; PSUM accumulates over dt. Invalid
    (height, t) positions get an additive -1e30 mask before the max-
    reduce. max(relu(x+b)) == relu(max(x)+b), so relu+bias happen after
    the max on the scalar engine.
  - conv weights stream as 28 tap-group DMAs (<=8 taps each) issued in
    the order the gather chase needs them, so the first chunks' weights
    land ~2µs after kernel start instead of ~30µs.
  - vertical conv never materialized: since o_v enters the fc linearly,
    G[t,d,k] = sum_v Wv[v,t]*fc_W[v*128+d,k] is precomputed on host and
    E @ G is added straight into the fc accumulation PSUM.
  - fc bias added via a K=1 ones-matmul.
"""

import math
import os
import sys
from contextlib import ExitStack

import numpy as np

sys.path.insert(0, "/opt/trn_rl_repo")

import ml_dtypes

import concourse.bass as bass
import concourse.tile as tile
from concourse import mybir
from concourse.bass import IndirectOffsetOnAxis
from concourse.bass_utils import run_bass_kernel_spmd
from concourse.masks import make_identity
from concourse.vector_clock import ScopedClock


def _patch_tile_drain():
    """This container's walrus codegen only accepts one sync-wait per Drain
    (CTRL_NO_STRUCT); Tile's kernel-tail drain carries one wait per live
    semaphore. Split the waits across a chain of drains, one wait each."""
    if getattr(tile.TileContext, "_drain_split_patched", False):
        return

    def _patched(self, tick_clock, wait_clock):
        nc = self.nc
        probe = nc.sync.drain()
        wait_clock.add_sem_waits(
            probe.ins, ScopedClock({None: tick_clock.global_clock}))
        nc.all_engine_barrier()
        popped = nc._tile_sem_poison_stack.pop()
        assert popped is self._sem_poison
        nc.clear_and_free_semaphores(list(self.sems.allocated().values()))
        nc.all_engine_barrier()

    tile.TileContext._drain_and_barrier = _patched
    tile.TileContext._drain_split_patched = True


_patch_tile_drain()


def _split_json_waits(j, max_waits=1):
    """This walrus codegen accepts at most one sync-wait per instruction.
    Hoist extra waits onto wait-only EventSemaphore instructions inserted
    just before the offender on the same engine queue."""
    n = 0
    for fn in j["functions"]:
        for blk in fn["blocks"]:
            out = []
            for inst in blk["instructions"]:
                si = inst.get("sync_info")
                waits = (si or {}).get("on_wait") or []
                if len(waits) > max_waits:
                    for k, w in enumerate(waits[:-max_waits]):
                        out.append({
                            "debug": inst.get("debug", 0),
                            "engine": inst["engine"],
                            "ins": [], "outs": [],
                            "name": f"{inst['name']}_wsplit{k}",
                            "opcode": "EventSemaphore",
                            "sync_info": {"on_update": [], "on_wait": [w]},
                        })
                        n += 1
                    si["on_wait"] = waits[-max_waits:]
                out.append(inst)
            blk["instructions"] = out
    return n


def _install_wait_splitter(nc):
    import json as _json

    orig = nc.to_json_bytes

    def patched():
        j = _json.loads(orig())
        _split_json_waits(j)
        return _json.dumps(j).encode()

    nc.to_json_bytes = patched

B = 1024
L = 50
D = 128
NV = 8
NH = 16
NU = 100000
NI = 500000
NCORES = 8
BLOC = B // NCORES          # 128 batch rows per core
TP = 64                     # t' pitch in ET (>= max dt + max Nt = 56)
NEG = -1.0e30
FC_IN = NV * D + NH * L     # 1824
NOUT = 2 * D                # 256
WGRP = 8                    # taps per weight-DMA group

# Height-chunk table: heights [8u, 8u+nh) packed as m2 = 16*(i-8u)+j.
# ndt taps accumulate in PSUM; Nt is the t-window (valid-t of the chunk's
# shortest filter); Nb batch rows per matmul so that Nb*Nt <= 512.
CHUNKS = []
_base = 0
for _u in range(7):
    _i0 = 8 * _u
    _nh = min(8, L - _i0)
    _ndt = min(_i0 + 8, L)
    _nt = L - _i0
    _nb = min(BLOC, 512 // _nt)
    _nblk = math.ceil(BLOC / _nb)
    CHUNKS.append(dict(i0=_i0, nh=_nh, ndt=_ndt, nt=_nt, nb=_nb,
                       nblk=_nblk, base=_base))
    _base += _ndt
NWTILES = _base             # 218 weight tiles of [d=128, m2=128]

# Weight-DMA groups: chunk u's taps split into ceil(ndt/WGRP) slices;
# group (u, k) covers taps [k*WGRP, min((k+1)*WGRP, ndt)). It is first
# needed when tap k*WGRP becomes runnable, i.e. at gather column
# min(k*WGRP + nt - 1, L-1). Stream the DMAs in need order.
WGROUPS = []                # (need_col, u, k, dt0, dt1)
for _u, _ch in enumerate(CHUNKS):
    for _k in range(math.ceil(_ch["ndt"] / WGRP)):
        _dt0 = _k * WGRP
        _dt1 = min(_dt0 + WGRP, _ch["ndt"])
        _need = min(_dt0 + _ch["nt"] - 1, L - 1)
        WGROUPS.append((_need, _u, _k, _dt0, _dt1))
WGROUPS.sort(key=lambda g: (g[0], -g[1]))
# trigger each group's DMA a few gather slots before it is needed
WTRIG = {}
for _g in WGROUPS:
    WTRIG.setdefault(max(0, _g[0] - 6), []).append(_g)

_NC_CACHE = None

# Set BASS_KERNEL_TRACE=1 to profile; exec time lands in LAST_RESULTS.
LAST_RESULTS = None


def _build_nc():
    f32 = mybir.dt.float32
    bf16 = mybir.dt.bfloat16
    i32 = mybir.dt.int32
    X = mybir.AxisListType.X
    Copy = mybir.ActivationFunctionType.Copy

    nc = bass.Bass()
    seq_t = nc.dram_tensor("seq_idx", [BLOC, L], i32, kind="ExternalInput")
    uid_t = nc.dram_tensor("uid_idx", [BLOC, 1], i32, kind="ExternalInput")
    item_t = nc.dram_tensor("item_emb", [NI, D], f32, kind="ExternalInput")
    user_t = nc.dram_tensor("user_emb", [NU, D], f32, kind="ExternalInput")
    whp_t = nc.dram_tensor("whp", [D, NWTILES * 128], bf16, kind="ExternalInput")
    g_t = nc.dram_tensor("g", [D, L * D], bf16, kind="ExternalInput")
    fcwh_t = nc.dram_tensor("fcwh", [128, 7 * D], bf16, kind="ExternalInput")
    masks_t = nc.dram_tensor("masks", [128, 7 * 512], f32, kind="ExternalInput")
    bh_t = nc.dram_tensor("bh_p", [128, 7], f32, kind="ExternalInput")
    fcb_t = nc.dram_tensor("fcb", [1, D], bf16, kind="ExternalInput")
    out_t = nc.dram_tensor("out", [BLOC, NOUT], f32, kind="ExternalOutput")

    # conv matmul (u, blk, dt) becomes runnable once ET column
    # min(dt + Nt - 1, L-1) is gathered (t' >= L is the zero pad).
    # These six groups chase the gather stream; the rest run after it.
    PHASE_A = [(6, 0), (5, 0), (5, 1), (5, 2), (4, 0), (4, 1)]

    with ExitStack() as ctx:
        tc = ctx.enter_context(tile.TileContext(nc))
        const = ctx.enter_context(tc.tile_pool(name="const", bufs=1))
        egath = ctx.enter_context(tc.tile_pool(name="egath", bufs=16))
        ebfp = ctx.enter_context(tc.tile_pool(name="ebfp", bufs=4))
        gpool = ctx.enter_context(tc.tile_pool(name="gpool", bufs=8))
        etp = ctx.enter_context(tc.tile_pool(name="etp", bufs=1))
        wpool = ctx.enter_context(tc.tile_pool(name="wpool", bufs=1))
        ohp = ctx.enter_context(tc.tile_pool(name="ohp", bufs=1))
        misc = ctx.enter_context(tc.tile_pool(name="misc", bufs=1))
        tpsum = ctx.enter_context(tc.tile_pool(name="tpsum", bufs=1, space="PSUM"))
        cpsum = ctx.enter_context(tc.tile_pool(name="cpsum", bufs=6, space="PSUM"))
        zpsum = ctx.enter_context(tc.tile_pool(name="zpsum", bufs=1, space="PSUM"))

        # --- constants (sync ring: small, then the 50 g slices) ---
        seq_sb = const.tile([BLOC, L], i32)
        nc.sync.dma_start(out=seq_sb[:], in_=seq_t[:])
        uid_sb = const.tile([BLOC, 1], i32)
        nc.sync.dma_start(out=uid_sb[:], in_=uid_t[:])
        ident = const.tile([128, 128], bf16)
        make_identity(nc, ident[:])
        fcwh_sb = const.tile([128, 7 * D], bf16)
        nc.sync.dma_start(out=fcwh_sb[:], in_=fcwh_t[:])
        mask_sb = const.tile([128, 7 * 512], f32)
        nc.sync.dma_start(out=mask_sb[:], in_=masks_t[:])
        bh_sb = const.tile([128, 7], f32)
        nc.sync.dma_start(out=bh_sb[:], in_=bh_t[:])
        fcb_sb = const.tile([1, D], bf16)
        nc.sync.dma_start(out=fcb_sb[:], in_=fcb_t[:])
        ones_sb = const.tile([1, BLOC], bf16)
        nc.vector.memset(ones_sb[:], 1.0)
        zline = const.tile([D, 1], bf16)
        nc.vector.memset(zline[:], 0.0)
        zfill = const.tile([128, BLOC], bf16)
        nc.vector.memset(zfill[:], 0.0)

        # --- conv weights: one resident tile, streamed per tap-group on the
        # scalar ring, in chase-need order (interleaved into the t-loop) ---
        whp_sb = wpool.tile([D, NWTILES * 128], bf16, name="whp_sb")

        def load_wgroup(g):
            _, u, _, dt0, dt1 = g
            base = CHUNKS[u]["base"]
            c0, c1 = (base + dt0) * 128, (base + dt1) * 128
            nc.scalar.dma_start(out=whp_sb[:, c0:c1], in_=whp_t[:, c0:c1])

        # --- ET[d, b, t'], zero pad for t' >= L ---
        et = etp.tile([D, BLOC, TP], bf16)
        nc.vector.tensor_copy(out=et[:, :, L:TP],
                              in_=zline[:].to_broadcast([D, BLOC, TP - L]))

        # --- fc accumulation PSUM [b, k]; group closes on last o_h matmul.
        # The opening bias matmul is emitted inside the t-loop (after the
        # first transpose) so a slow fcb load can't stall the PE queue head.
        zp = zpsum.tile([BLOC, D], f32)

        # conv emission bookkeeping
        chase = {}
        fc_pending = []
        for u, blk in PHASE_A:
            nt = CHUNKS[u]["nt"]
            for dt in range(CHUNKS[u]["ndt"]):
                chase.setdefault(min(dt + nt - 1, L - 1), []).append((u, blk, dt))
        psum_tiles = {}
        blocks_left = [ch["nblk"] for ch in CHUNKS]
        oh_tiles = {}

        def get_ohu(u):
            if u not in oh_tiles:
                oh_tiles[u] = ohp.tile([128, BLOC], bf16, tag=f"oh{u}",
                                       name=f"oh{u}")
            return oh_tiles[u]

        def emit_conv_mm(u, blk, dt):
            ch = CHUNKS[u]
            nt, nb, ndt = ch["nt"], ch["nb"], ch["ndt"]
            b0 = blk * nb
            nbb = min(nb, BLOC - b0)
            n = nbb * nt
            key = (u, blk)
            if key not in psum_tiles:
                while len(fc_pending) > 1:
                    uu = fc_pending.pop(0)
                    nc.tensor.matmul(out=zp[:], lhsT=oh_tiles[uu][:],
                                     rhs=fcwh_sb[:, uu * D:(uu + 1) * D],
                                     start=False, stop=False)
                psum_tiles[key] = cpsum.tile([128, 512], f32, tag="cps",
                                             name=f"cps_{u}_{blk}")
            ps = psum_tiles[key]
            # Tap dt only feeds heights i >= dt, whose valid t stops at
            # 49 - max(dt, i0): shrink the t-window for late taps. The
            # skipped cells are all masked to -1e30 before the reduce, so
            # the output is bit-identical. Saves ~4% of conv PE rows.
            nt_eff = min(nt, L - max(dt, ch["i0"]))
            out_ap = (ps[:, :n] if nt_eff == nt else
                      ps[:, :n].rearrange("p (b t) -> p b t", t=nt)[:, :, :nt_eff])
            nc.tensor.matmul(
                out=out_ap,
                lhsT=whp_sb[:, (ch["base"] + dt) * 128:(ch["base"] + dt + 1) * 128],
                rhs=et[:, b0:b0 + nbb, dt:dt + nt_eff],
                start=(dt == 0), stop=(dt == ndt - 1))
            if dt == ndt - 1:
                nc.vector.tensor_tensor(
                    out=ps[:, :n], in0=ps[:, :n],
                    in1=mask_sb[:, u * 512:u * 512 + n],
                    op=mybir.AluOpType.add)
                nc.vector.reduce_max(
                    out=get_ohu(u)[:, b0:b0 + nbb],
                    in_=ps[:, :n].rearrange("p (b t) -> p b t", t=nt),
                    axis=X)
                del psum_tiles[key]
                blocks_left[u] -= 1
                if blocks_left[u] == 0:
                    ohu = get_ohu(u)
                    nc.scalar.activation(ohu[:], ohu[:],
                                         mybir.ActivationFunctionType.Relu,
                                         bias=bh_sb[:, u:u + 1])
                    fc_pending.append(u)

        # --- the chase loop: gather -> bf16 cast -> transpose -> copy ->
        # G matmul, with ready conv matmuls interleaved into the PE stream
        # and weight-group DMAs interleaved into the scalar queue ---
        g_tiles = {}

        def emit_g_mm(t):
            nc.tensor.matmul(out=zp[:], lhsT=et[:, :, t], rhs=g_tiles.pop(t),
                             start=False, stop=False)

        for t in range(L):
            for g in WTRIG.get(t, ()):
                load_wgroup(g)
            e_t = egath.tile([BLOC, D], f32, tag="eg")
            nc.gpsimd.indirect_dma_start(
                out=e_t[:], out_offset=None, in_=item_t[:],
                in_offset=IndirectOffsetOnAxis(ap=seq_sb[:, t:t + 1], axis=0))
            ebf = ebfp.tile([BLOC, D], bf16, tag="ebf")
            nc.scalar.activation(ebf[:], e_t[:], Copy)
            tp = tpsum.tile([128, 128], bf16, tag="tp")
            nc.tensor.transpose(out=tp[:], in_=ebf[:], identity=ident[:])
            nc.vector.tensor_copy(out=et[:, :, t], in_=tp[:])
            gt = gpool.tile([D, D], bf16, tag="g", name=f"g{t}")
            nc.sync.dma_start(out=gt[:], in_=g_t[:, t * D:(t + 1) * D])
            g_tiles[t] = gt
            if t == 0:
                nc.tensor.matmul(out=zp[:], lhsT=ones_sb[:], rhs=fcb_sb[:],
                                 start=True, stop=False)
            else:
                emit_g_mm(t - 1)
            for (u, blk, dt) in chase.get(t, ()):
                emit_conv_mm(u, blk, dt)
            # Zero-contribution fillers (exact +0 into the open zp group)
            # keep the PE gaplessly busy through the gather window so HAM
            # ramps to full clock early and stays there.
            if t >= 1:
                nfill = 12 if t < 9 else (6 if t < 17 else 4)
                for _ in range(nfill):
                    nc.tensor.matmul(out=zp[:], lhsT=zfill[:],
                                     rhs=ident[:], start=False, stop=False)
        emit_g_mm(L - 1)

        # --- the P_u gather (off the gather window's critical path) ---
        pu_sb = misc.tile([BLOC, D], f32, tag="pu")
        nc.gpsimd.indirect_dma_start(
            out=pu_sb[:], out_offset=None, in_=user_t[:],
            in_offset=IndirectOffsetOnAxis(ap=uid_sb[:, :1], axis=0))
        nc.sync.dma_start(out=out_t[:, D:NOUT], in_=pu_sb[:])

        # --- remaining conv chunks, block-sequential: each block's reduce
        # overlaps the next block's matmuls on a different PSUM bank ---
        done_a = set(PHASE_A)
        for u in [4, 3, 2, 1, 0]:
            for blk in range(CHUNKS[u]["nblk"]):
                if (u, blk) in done_a:
                    continue
                for dt in range(CHUNKS[u]["ndt"]):
                    emit_conv_mm(u, blk, dt)

        # --- remaining o_h fc matmuls ---
        for i, u in enumerate(fc_pending):
            nc.tensor.matmul(out=zp[:], lhsT=oh_tiles[u][:],
                             rhs=fcwh_sb[:, u * D:(u + 1) * D],
                             start=False, stop=(i == len(fc_pending) - 1))

        z_sb = misc.tile([BLOC, D], f32, tag="z")
        nc.scalar.activation(z_sb[:], zp[:], mybir.ActivationFunctionType.Relu)
        nc.sync.dma_start(out=out_t[:, 0:D], in_=z_sb[:])

    return nc


def _prep_common(user_emb, item_emb, Wv, bv, Wh, bh, fc_W, fc_b):
    f = np.float32
    b16 = ml_dtypes.bfloat16
    item_emb = np.ascontiguousarray(np.asarray(item_emb, f))
    user_emb = np.ascontiguousarray(np.asarray(user_emb, f))
    Wh = np.asarray(Wh, f)          # [L, NH, L, D], zero for dt > i
    Wv = np.asarray(Wv, f)          # [NV, L]
    bv = np.asarray(bv, f)
    bh = np.asarray(bh, f)          # [L, NH]
    fc_W = np.asarray(fc_W, f)      # [FC_IN, D]
    fc_b = np.asarray(fc_b, f)

    whp = np.zeros((D, NWTILES * 128), f)
    masks = np.full((128, 7 * 512), 0.0, f)
    fcwh = np.zeros((128, 7 * D), f)
    bh_p = np.zeros((128, 7), f)
    fcw_h = fc_W[NV * D:]           # [800, D]
    for u, ch in enumerate(CHUNKS):
        i0, nh, ndt, nt, nb = ch["i0"], ch["nh"], ch["ndt"], ch["nt"], ch["nb"]
        base = ch["base"]
        wu = Wh[i0:i0 + nh]         # [nh, NH, L, D]
        for dt in range(ndt):
            blkw = wu[:, :, dt, :].reshape(nh * NH, D)
            whp[:, (base + dt) * 128:(base + dt) * 128 + nh * NH] = blkw.T
        m = np.full((128, nb * nt), NEG, f)
        for mm in range(nh * NH):
            i = i0 + mm // NH
            vt = min(L - i, nt)
            row = np.full((nt,), NEG, f)
            row[:vt] = 0.0
            m[mm] = np.tile(row, nb)
        masks[:, u * 512:u * 512 + nb * nt] = m
        fcwh[:nh * NH, u * D:(u + 1) * D] = fcw_h[u * 128:u * 128 + nh * NH]
        bh_p[:nh * NH, u] = bh[i0:i0 + nh].reshape(nh * NH)

    fcv = fc_W[:NV * D].reshape(NV, D, D)
    g = np.einsum("vt,vdk->tdk", Wv, fcv)            # [L, D, D]
    g = np.ascontiguousarray(g.transpose(1, 0, 2).reshape(D, L * D))
    fcb = (fc_b + np.einsum("v,vdk->k", bv, fcv)).reshape(1, D)

    return dict(item_emb=item_emb, user_emb=user_emb,
                whp=whp.astype(b16), g=g.astype(b16),
                fcwh=fcwh.astype(b16), masks=masks, bh_p=bh_p,
                fcb=fcb.astype(b16))


def make_in_maps(user_ids, item_seq, user_emb, item_emb, Wv, bv, Wh, bh,
                 fc_W, fc_b):
    common = _prep_common(user_emb, item_emb, Wv, bv, Wh, bh, fc_W, fc_b)
    user_ids = np.asarray(user_ids).astype(np.int32).reshape(B, 1)
    item_seq = np.asarray(item_seq).astype(np.int32).reshape(B, L)
    in_maps = []
    for c in range(NCORES):
        m = dict(common)
        m["seq_idx"] = np.ascontiguousarray(item_seq[c * BLOC:(c + 1) * BLOC])
        m["uid_idx"] = np.ascontiguousarray(user_ids[c * BLOC:(c + 1) * BLOC])
        in_maps.append(m)
    return in_maps


def get_nc():
    global _NC_CACHE
    if _NC_CACHE is None:
        _NC_CACHE = _build_nc()
        _install_wait_splitter(_NC_CACHE)
    return _NC_CACHE


def kernel(**inputs) -> np.ndarray:
    global LAST_RESULTS
    in_maps = make_in_maps(**inputs)
    nc = get_nc()
    trace = bool(int(os.environ.get("BASS_KERNEL_TRACE", "0")))
    res = run_bass_kernel_spmd(nc, in_maps, list(range(NCORES)), trace=trace)
    LAST_RESULTS = res
    return np.concatenate([res.results[c]["out"] for c in range(NCORES)], axis=0)
